# revision 7
# baseline (speedup 1.0000x reference)
"""MoE (top-2 of 8 experts, d=1024) — expert-parallel Bass kernel for 8 trn2 cores.

Strategy: expert-parallel with fixed-capacity overflow slots, fp16 datapath.

Each core c owns expert c: a primary token block of fixed capacity P
(chunked [512, 512, ..., rem]) plus ONE fixed-size secondary slot of S
tokens that holds overflow from any single (possibly different) expert,
whose weights are shipped as a second weight set. (P, S) are chosen on
the host so that all overflow pieces fit in the 8 secondary slots; the
program is identical across cores (SPMD), only the data differs. This
cuts per-core matmul columns from pad256(max_count) to ~N/8 + S.

Device computes only  y = relu(x@W1 + b1) @ W2  in fp16 (full PE rate,
half the HBM/SBUF traffic of f32r); the combine weight w and the b2 bias
are applied on the host during the scatter-add combine:
    out[tok] += w * (y + b2[e]).

Device-side details:
 - software pipeline at mc granularity: h-phase of chunk n interleaves
   with y-phase of chunk n-1, so the PE never waits for the relu tail
 - relu+bias on the scalar engine (ACT), PSUM->SBUF fp16 copy on DVE:
   each engine ~20% busy, off the PE critical path
 - per-kc weight tiles so next iteration's weight reload streams in
   progressively as soon as each slice's last reader retires
"""

import numpy as np

import concourse.bass as bass
import concourse.mybir as mybir
import concourse.tile as tile
from concourse import bacc
from concourse.bass_utils import run_bass_kernel_spmd

# Problem shapes (hardcoded per contract)
D = 1024  # d_model == d_hidden
N_EXPERTS = 8
TOP_K = 2
N_CORES = 8
B, T = 4, 2048
N_TOKENS = B * T

F16 = mybir.dt.float16
F32 = mybir.dt.float32
KC = D // 128  # contraction chunks (8)
MC = D // 128  # output-feature chunks (8)
NT = 512      # max tokens per matmul / chunk


def plan_capacity(counts):
    """Choose primary capacity P and secondary slot size S.

    Every core runs P primary tokens (its own expert, zero-padded) plus one
    S-token slot taking a single contiguous overflow piece of one expert.
    Feasible iff the overflow (count-P clamped) splits into <= N_CORES
    pieces of size <= S. Minimize modeled PE cycles 128*P + 128*max(60,S+6).
    """
    best = None
    for P in range(1024, 2 * N_TOKENS + 512, 128):
        ov = [max(0, c - P) for c in counts]
        tot = sum(ov)
        if tot == 0:
            cost = 128 * P
            if best is None or cost < best[0]:
                best = (cost, P, 0)
            break
        for S in range(32, 513, 32):
            pieces = sum(-(-o // S) for o in ov)
            if pieces <= N_CORES:
                cost = 128 * P + 128 * max(60, S + 6)
                if best is None or cost < best[0]:
                    best = (cost, P, S)
                break  # larger S only costs more at this P
    assert best is not None, f"no feasible capacity plan for counts={counts}"
    return best[1], best[2]


def chunk_sizes(P):
    sizes = [NT] * (P // NT)
    if P % NT:
        sizes.append(P % NT)
    return sizes


def build_moe_kernel(P: int, S: int, repeat: int = 1,
                     act_relu: bool = True) -> bacc.Bacc:
    """yP = relu(xP@w1a + b1a)@w2a ; yS = relu(xS@w1b + b1b)@w2b (fp16)."""
    sizes_p = chunk_sizes(P)
    all_sizes = sizes_p + ([S] if S else [])
    nchk = len(all_sizes)
    n_primary = len(sizes_p)
    offs = [sum(all_sizes[:i]) for i in range(nchk)]  # offset within own tensor
    p_offs = offs[:n_primary]

    nc = bacc.Bacc("TRN2", target_bir_lowering=False, debug=False,
                   num_devices=N_CORES)

    xP = nc.dram_tensor("xP", [D, P], F16, kind="ExternalInput")
    w1a = nc.dram_tensor("w1a", [D, D], F16, kind="ExternalInput")
    w2a = nc.dram_tensor("w2a", [D, D], F16, kind="ExternalInput")
    b1a = nc.dram_tensor("b1a", [D], F32, kind="ExternalInput")
    yP = nc.dram_tensor("yP", [D, P], F16, kind="ExternalOutput")
    if S:
        xS = nc.dram_tensor("xS", [D, S], F16, kind="ExternalInput")
        w1b = nc.dram_tensor("w1b", [D, D], F16, kind="ExternalInput")
        w2b = nc.dram_tensor("w2b", [D, D], F16, kind="ExternalInput")
        b1b = nc.dram_tensor("b1b", [D], F32, kind="ExternalInput")
        yS = nc.dram_tensor("yS", [D, S], F16, kind="ExternalOutput")

    xP_v = xP.ap().rearrange("(kc kp) t -> kp kc t", kc=KC)
    yP_v = yP.ap().rearrange("(mc mp) t -> mp mc t", mc=MC)
    w1a_v = w1a.ap().rearrange("(kc kp) m -> kp kc m", kc=KC)
    w2a_v = w2a.ap().rearrange("(kc kp) m -> kp kc m", kc=KC)
    b1a_v = b1a.ap().rearrange("(mc mp) -> mp mc", mc=MC)
    if S:
        xS_v = xS.ap().rearrange("(kc kp) t -> kp kc t", kc=KC)
        yS_v = yS.ap().rearrange("(mc mp) t -> mp mc t", mc=MC)
        w1b_v = w1b.ap().rearrange("(kc kp) m -> kp kc m", kc=KC)
        w2b_v = w2b.ap().rearrange("(kc kp) m -> kp kc m", kc=KC)
        b1b_v = b1b.ap().rearrange("(mc mp) -> mp mc", mc=MC)

    def x_view(n):
        if n < n_primary:
            return xP_v[:, :, p_offs[n]:p_offs[n] + all_sizes[n]]
        return xS_v

    def y_view(n, mc):
        if n < n_primary:
            return yP_v[:, mc, p_offs[n]:p_offs[n] + all_sizes[n]]
        return yS_v[:, mc, :]

    with tile.TileContext(nc) as tc:
        with (
            tc.tile_pool(name="weights", bufs=1) as wpool,
            tc.tile_pool(name="consts", bufs=1) as cpool,
            tc.tile_pool(name="xin", bufs=3) as xpool,
            tc.tile_pool(name="hmid", bufs=2) as hpool,
            tc.tile_pool(name="yout", bufs=2) as ypool,
            tc.tile_pool(name="ph", bufs=3, space="PSUM") as phpool,
            tc.tile_pool(name="py", bufs=3, space="PSUM") as pypool,
        ):
            from contextlib import nullcontext
            loop_cm = (
                tc.For_i(0, repeat, 1,
                         hint_engines=(mybir.EngineType.PE,
                                       mybir.EngineType.Activation,
                                       mybir.EngineType.DVE,
                                       mybir.EngineType.SP))
                if repeat > 1 else nullcontext()
            )
            with loop_cm:
                # weight tiles, per-kc so next-iteration reloads stream in
                # as each slice's last reader retires
                w1a_t = [wpool.tile([128, D], F16, tag=f"w1a{k}", name=f"w1a{k}")
                         for k in range(KC)]
                b1a_sb = cpool.tile([128, MC], F32, tag="b1a")
                x0 = xpool.tile([128, KC, NT], F16, tag="x")
                for kc in range(KC):
                    nc.sync.dma_start(x0[:, kc, :all_sizes[0]],
                                      x_view(0)[:, kc, :])
                for k in range(KC):
                    nc.sync.dma_start(w1a_t[k][:], w1a_v[:, k, :])
                nc.sync.dma_start(b1a_sb[:], b1a_v)
                w2a_t = [wpool.tile([128, D], F16, tag=f"w2a{k}", name=f"w2a{k}")
                         for k in range(KC)]
                if S:
                    w1b_t = [wpool.tile([128, D], F16, tag=f"w1b{k}", name=f"w1b{k}")
                             for k in range(KC)]
                    w2b_t = [wpool.tile([128, D], F16, tag=f"w2b{k}", name=f"w2b{k}")
                             for k in range(KC)]
                    b1b_sb = cpool.tile([128, MC], F32, tag="b1b")

                # later-needed weight DMAs are emitted inside the pipeline
                # (after the x prefetch of the stage before their first use)
                # so they don't queue ahead of x chunks on the DMA engines
                def emit_late_dma(s):
                    if s == 0:
                        for k in range(KC):
                            nc.sync.dma_start(w2a_t[k][:], w2a_v[:, k, :])
                    elif s == 1 and S:
                        for k in range(KC):
                            nc.sync.dma_start(w1b_t[k][:], w1b_v[:, k, :])
                        nc.sync.dma_start(b1b_sb[:], b1b_v)
                    elif s == 2 and S:
                        for k in range(KC):
                            nc.sync.dma_start(w2b_t[k][:], w2b_v[:, k, :])

                def wset(n):
                    if n < n_primary:
                        return w1a_t, w2a_t, b1a_sb
                    return w1b_t, w2b_t, b1b_sb

                def emit_x_dma(n, x_sb):
                    for kc in range(KC):
                        nc.sync.dma_start(x_sb[:, kc, :all_sizes[n]],
                                          x_view(n)[:, kc, :])

                def emit_h_mc(n, mc, x_sb, h_sb):
                    sz = all_sizes[n]
                    w1_t, _, b1_sb = wset(n)
                    ph = phpool.tile([128, NT], F32, tag="ph", name=f"ph{n}_{mc}")
                    for kc in range(KC):
                        nc.tensor.matmul(
                            ph[:, :sz],
                            w1_t[kc][:, bass.ts(mc, 128)],
                            x_sb[:, kc, :sz],
                            start=(kc == 0), stop=(kc == KC - 1),
                        )
                    if act_relu:
                        nc.scalar.activation(
                            h_sb[:, mc, :sz], ph[:, :sz],
                            mybir.ActivationFunctionType.Relu,
                            bias=b1_sb[:, mc:mc + 1],
                        )
                    else:
                        nc.vector.tensor_scalar(
                            h_sb[:, mc, :sz], ph[:, :sz],
                            b1_sb[:, mc:mc + 1], 0.0,
                            mybir.AluOpType.add, mybir.AluOpType.max,
                        )

                def emit_y_mc(n, mc, h_sb, y_sb):
                    sz = all_sizes[n]
                    _, w2_t, _ = wset(n)
                    py = pypool.tile([128, NT], F32, tag="py", name=f"py{n}_{mc}")
                    for kc in range(KC):
                        nc.tensor.matmul(
                            py[:, :sz],
                            w2_t[kc][:, bass.ts(mc, 128)],
                            h_sb[:, kc, :sz],
                            start=(kc == 0), stop=(kc == KC - 1),
                        )
                    nc.vector.tensor_copy(y_sb[:, mc, :sz], py[:, :sz])
                    nc.sync.dma_start(y_view(n, mc), y_sb[:, mc, :sz])

                # software pipeline: h-phase(s) interleaves with y-phase(s-1)
                x_tiles = {0: x0}
                h_tiles = {}
                y_tiles = {}
                for s in range(nchk + 1):
                    if s + 1 < nchk:
                        xt = xpool.tile([128, KC, NT], F16, tag="x")
                        emit_x_dma(s + 1, xt)
                        x_tiles[s + 1] = xt
                    emit_late_dma(s)
                    if s < nchk:
                        h_tiles[s] = hpool.tile([128, KC, NT], F16, tag="h", name=f"h{s}")
                    if s > 0:
                        y_tiles[s - 1] = ypool.tile([128, MC, NT], F16,
                                                    tag="y", name=f"y{s-1}")
                    for mc in range(MC):
                        if s < nchk:
                            emit_h_mc(s, mc, x_tiles[s], h_tiles[s])
                        if s > 0:
                            emit_y_mc(s - 1, mc, h_tiles[s - 1],
                                      y_tiles[s - 1])
                    x_tiles.pop(s - 1, None)
                    h_tiles.pop(s - 2, None)
                    y_tiles.pop(s - 2, None)

    nc.compile()
    return nc


_NC_CACHE: dict = {}


def _get_kernel(P: int, S: int, repeat: int = 1, **opts) -> bacc.Bacc:
    key = (P, S, repeat, tuple(sorted(opts.items())))
    if key not in _NC_CACHE:
        _NC_CACHE[key] = build_moe_kernel(P, S, repeat, **opts)
    return _NC_CACHE[key]


def dispatch(x, W_gate, b_gate):
    """Host-side gate + top-2 dispatch plan.

    Returns (xf, plan) where plan has per-core primary (expert c) token ids
    and weights plus at most one secondary (expert, ids, wts) piece.
    """
    xf = np.ascontiguousarray(np.asarray(x).reshape(-1, D), dtype=np.float32)
    scores = xf @ np.asarray(W_gate, np.float32) + np.asarray(b_gate, np.float32)
    top2 = np.argpartition(scores, N_EXPERTS - TOP_K, axis=1)[:, -TOP_K:]
    ids, wts = [], []
    for e in range(N_EXPERTS):
        tok = np.nonzero((top2 == e).any(axis=1))[0]
        ids.append(tok)
        wts.append(scores[tok, e])
    counts = [len(t) for t in ids]
    P, S = plan_capacity(counts)
    prim = [(ids[e][:P], wts[e][:P]) for e in range(N_EXPERTS)]
    pieces = []  # (expert, tok_ids, wts)
    for e in range(N_EXPERTS):
        for lo in range(P, counts[e], S):
            hi = min(lo + S, counts[e])
            pieces.append((e, ids[e][lo:hi], wts[e][lo:hi]))
    assert len(pieces) <= N_CORES
    sec = pieces + [None] * (N_CORES - len(pieces))
    return xf, {"P": P, "S": S, "prim": prim, "sec": sec}


def make_in_maps(parts, xf, plan):
    """Per-core input dicts (fp16 xT blocks + two weight sets)."""
    W1, b1, W2, _b2 = parts
    P, S = plan["P"], plan["S"]
    W1h = W1.astype(np.float16)
    W2h = W2.astype(np.float16)
    in_maps = []
    for c in range(N_CORES):
        tok, _w = plan["prim"][c]
        xPc = np.zeros((D, P), np.float16)
        xPc[:, :len(tok)] = xf[tok].astype(np.float16).T
        m = {
            "xP": xPc,
            "w1a": np.ascontiguousarray(W1h[c]),
            "w2a": np.ascontiguousarray(W2h[c]),
            "b1a": b1[c].astype(np.float32),
        }
        if S:
            piece = plan["sec"][c]
            e2 = piece[0] if piece else c
            xSc = np.zeros((D, S), np.float16)
            if piece:
                xSc[:, :len(piece[1])] = xf[piece[1]].astype(np.float16).T
            m.update({
                "xS": xSc,
                "w1b": np.ascontiguousarray(W1h[e2]),
                "w2b": np.ascontiguousarray(W2h[e2]),
                "b1b": b1[e2].astype(np.float32),
            })
        in_maps.append(m)
    return in_maps


def combine(res_list, plan, b2):
    """Scatter-add device outputs: out[tok] += w * (y + b2[e])."""
    out = np.zeros((N_TOKENS, D), np.float32)
    for c in range(N_CORES):
        tok, w = plan["prim"][c]
        yPc = np.asarray(res_list[c]["yP"], np.float32)  # [D, P]
        out[tok] += w[:, None] * (yPc.T[:len(tok)] + b2[c][None, :])
        if plan["S"]:
            piece = plan["sec"][c]
            if piece:
                e2, tok2, w2_ = piece
                ySc = np.asarray(res_list[c]["yS"], np.float32)
                out[tok2] += w2_[:, None] * (ySc.T[:len(tok2)] + b2[e2][None, :])
    return out


def kernel(x, W_gate, b_gate, W1, b1, W2, b2):
    xf, plan = dispatch(x, W_gate, b_gate)
    nc = _get_kernel(plan["P"], plan["S"])

    W1 = np.asarray(W1, np.float32)
    W2 = np.asarray(W2, np.float32)
    b1 = np.asarray(b1, np.float32)
    b2 = np.asarray(b2, np.float32)
    in_maps = make_in_maps((W1, b1, W2, b2), xf, plan)

    res = run_bass_kernel_spmd(nc, in_maps, core_ids=list(range(N_CORES)))

    out = combine(res.results, plan, b2)
    return out.reshape(B, T, D)
